# revision 5
# baseline (speedup 1.0000x reference)
"""Trainium2 Bass kernel for a fused CustomLSTMCell.

Math (reference):
    combined = concat([input, hidden], axis=1)            # [B, D], D = 2048
    gates    = combined @ concat([W_i,W_f,W_o,W_g], 1) + b  # [B, 4H]
    i, f, o, g = split(gates, 4, axis=1)
    new_cell   = sigmoid(f) * cell_state + sigmoid(i) * tanh(g)
    new_hidden = sigmoid(o) * tanh(new_cell)

Strategy:
  - Data-parallel over batch: 8 cores x 1024 rows each. No collectives.
  - Host prepares A^T = combined.T (contraction dim D on partitions) in bf16
    and W in bf16; PSUM accumulates in fp32.
  - Per core GEMM: [1024, 2048] @ [2048, 4096] as 128x128x512 matmul tiles;
    the stationary operand is the A^T tile (reused across 4 gate tiles per
    K step), the moving operand is W.
  - Gate columns are processed in (i,f,o,g)-aligned groups of 512 so the
    elementwise LSTM math is local to one [128 x 512] tile set.
  - Bias add on DVE (PSUM + replicated-bias SBUF tile), sigmoid/tanh on ACT,
    cell/hidden updates on DVE, everything overlapped with the PE stream by
    the Tile scheduler.
"""

import sys

if "/opt/trn_rl_repo" not in sys.path:
    sys.path.insert(0, "/opt/trn_rl_repo")

import ml_dtypes
import numpy as np

import concourse.bass as bass
import concourse.mybir as mybir
import concourse.tile as tile
from concourse import bacc
from concourse.bass_utils import run_bass_kernel_spmd

N_CORES = 8
B = 8192
IN_SIZE = 1024
H = 1024
D = IN_SIZE + H          # 2048 contraction dim
G4 = 4 * H               # 4096 gate columns
BC = B // N_CORES        # 1024 batch rows per core
P = 128                  # partitions
KT = D // P              # 16 k-tiles
MT = BC // P             # 8 m-tiles (batch row tiles) per core
NG = 512                 # gate columns processed per group (per gate)
HGRPS = H // NG          # 2 column groups

_NC_CACHE = {}


def _build(iters: int = 1) -> bass.Bass:
    nc = bacc.Bacc("TRN2", target_bir_lowering=False, debug=False)

    at = nc.dram_tensor("at", [D, BC], mybir.dt.bfloat16, kind="ExternalInput")
    w = nc.dram_tensor("w", [D, G4], mybir.dt.bfloat16, kind="ExternalInput")
    br = nc.dram_tensor("br", [P, G4], mybir.dt.float32, kind="ExternalInput")
    cs = nc.dram_tensor("cs", [BC, H], mybir.dt.float32, kind="ExternalInput")
    nh = nc.dram_tensor("nh", [BC, H], mybir.dt.float32, kind="ExternalOutput")
    ncl = nc.dram_tensor("ncl", [BC, H], mybir.dt.float32, kind="ExternalOutput")

    at_r = at.rearrange("(ko ki) b -> ki ko b", ki=P)   # [128, KT, BC]
    w_r = w.rearrange("(ko ki) n -> ki ko n", ki=P)     # [128, KT, G4]
    cs_r = cs.rearrange("(m p) h -> m p h", p=P)        # [MT, 128, H]
    nh_r = nh.rearrange("(m p) h -> m p h", p=P)
    ncl_r = ncl.rearrange("(m p) h -> m p h", p=P)

    AF = mybir.ActivationFunctionType

    with tile.TileContext(nc) as tc:
        with (
            tc.tile_pool(name="resident", bufs=1) as rpool,
            tc.tile_pool(name="work", bufs=3) as wpool,
            tc.tile_pool(name="psum", bufs=2, space="PSUM") as ppool,
        ):
            for _ in range(iters):
                at_sb = rpool.tile([P, KT, BC], mybir.dt.bfloat16, tag="at")
                w_sb = rpool.tile([P, KT, G4], mybir.dt.bfloat16, tag="w")
                br_sb = rpool.tile([P, G4], mybir.dt.float32, tag="br")
                nc.sync.dma_start(out=br_sb[:], in_=br[:])
                for k in range(KT):
                    nc.sync.dma_start(out=at_sb[:, k, :], in_=at_r[:, k, :])
                    nc.sync.dma_start(out=w_sb[:, k, :], in_=w_r[:, k, :])

                for m in range(MT):
                    for g in range(HGRPS):
                        # column starts for i, f, o, g slices of this h-group
                        cols = [q * H + g * NG for q in range(4)]
                        ps = [
                            ppool.tile([P, NG], mybir.dt.float32, tag=f"ps{q}", name=f"ps{q}")
                            for q in range(4)
                        ]
                        lhs0 = at_sb[:, 0, m * P : (m + 1) * P]
                        for k in range(KT):
                            lhs = at_sb[:, k, m * P : (m + 1) * P]
                            for q in range(4):
                                nc.tensor.matmul(
                                    ps[q][:],
                                    lhsT=lhs,
                                    rhs=w_sb[:, k, cols[q] : cols[q] + NG],
                                    start=(k == 0),
                                    stop=(k == KT - 1),
                                )
                        # bias add (DVE) + activation (ACT), PSUM -> SBUF
                        gt = [
                            wpool.tile([P, NG], mybir.dt.float32, tag=f"gt{q}", name=f"gt{q}")
                            for q in range(4)
                        ]
                        for q in range(4):
                            nc.vector.tensor_add(
                                out=gt[q][:],
                                in0=ps[q][:],
                                in1=br_sb[:, cols[q] : cols[q] + NG],
                            )
                        for q in range(3):
                            nc.scalar.activation(gt[q][:], gt[q][:], AF.Sigmoid)
                        nc.scalar.activation(gt[3][:], gt[3][:], AF.Tanh)

                        cl = wpool.tile([P, NG], mybir.dt.float32, tag="cl")
                        nc.sync.dma_start(
                            out=cl[:], in_=cs_r[m, :, g * NG : (g + 1) * NG]
                        )
                        si, sf, so, sg = gt
                        # new_cell = sigmoid(f)*c + sigmoid(i)*tanh(g)  -> sf
                        nc.vector.tensor_mul(out=sf[:], in0=sf[:], in1=cl[:])
                        nc.vector.tensor_mul(out=si[:], in0=si[:], in1=sg[:])
                        nc.vector.tensor_add(out=sf[:], in0=sf[:], in1=si[:])
                        # new_hidden = sigmoid(o)*tanh(new_cell)        -> so
                        nc.scalar.activation(sg[:], sf[:], AF.Tanh)
                        nc.vector.tensor_mul(out=so[:], in0=so[:], in1=sg[:])
                        nc.sync.dma_start(
                            out=ncl_r[m, :, g * NG : (g + 1) * NG], in_=sf[:]
                        )
                        nc.sync.dma_start(
                            out=nh_r[m, :, g * NG : (g + 1) * NG], in_=so[:]
                        )
    nc.finalize()
    return nc


def get_nc(iters: int = 1) -> bass.Bass:
    if iters not in _NC_CACHE:
        _NC_CACHE[iters] = _build(iters)
    return _NC_CACHE[iters]


def make_in_maps(input, hidden, cell_state, W_i, b_i, W_f, b_f, W_o, b_o, W_g, b_g):
    comb = np.concatenate(
        [np.asarray(input, np.float32), np.asarray(hidden, np.float32)], axis=1
    )  # [B, D]
    W = np.concatenate(
        [np.asarray(W_i), np.asarray(W_f), np.asarray(W_o), np.asarray(W_g)], axis=1
    ).astype(np.float32)  # [D, 4H]
    b = np.concatenate(
        [np.asarray(b_i), np.asarray(b_f), np.asarray(b_o), np.asarray(b_g)]
    ).astype(np.float32)  # [4H]

    at_full = comb.T.astype(ml_dtypes.bfloat16)  # [D, B]
    w_bf = np.ascontiguousarray(W.astype(ml_dtypes.bfloat16))
    br = np.ascontiguousarray(np.broadcast_to(b, (P, G4)))
    cs = np.asarray(cell_state, np.float32)

    in_maps = []
    for c in range(N_CORES):
        sl = slice(c * BC, (c + 1) * BC)
        in_maps.append(
            {
                "at": np.ascontiguousarray(at_full[:, sl]),
                "w": w_bf,
                "br": br,
                "cs": np.ascontiguousarray(cs[sl]),
            }
        )
    return in_maps


def kernel(input, hidden, cell_state, W_i, b_i, W_f, b_f, W_o, b_o, W_g, b_g):
    in_maps = make_in_maps(
        input, hidden, cell_state, W_i, b_i, W_f, b_f, W_o, b_o, W_g, b_g
    )
    nc = get_nc(1)
    res = run_bass_kernel_spmd(nc, in_maps, core_ids=list(range(N_CORES)))
    new_hidden = np.concatenate(
        [res.results[c]["nh"] for c in range(N_CORES)], axis=0
    )
    new_cell = np.concatenate(
        [res.results[c]["ncl"] for c in range(N_CORES)], axis=0
    )
    return new_hidden, new_cell


# revision 6
# speedup vs baseline: 6.4785x; 6.4785x over previous
"""Trainium2 Bass kernel for a fused CustomLSTMCell.

Math (reference):
    combined = concat([input, hidden], axis=1)            # [B, D], D = 2048
    gates    = combined @ concat([W_i,W_f,W_o,W_g], 1) + b  # [B, 4H]
    i, f, o, g = split(gates, 4, axis=1)
    new_cell   = sigmoid(f) * cell_state + sigmoid(i) * tanh(g)
    new_hidden = sigmoid(o) * tanh(new_cell)

Strategy:
  - Data-parallel over batch: 8 cores x 1024 rows each. No collectives.
  - Host prepares A^T = combined.T (contraction dim D on partitions) in bf16
    and W in bf16; PSUM accumulates in fp32.
  - Per core GEMM: [1024, 2048] @ [2048, 4096] as 128x128x512 matmul tiles;
    the stationary operand is the A^T tile (reused across 4 gate tiles per
    K step), the moving operand is W.
  - Gate columns are processed in (i,f,o,g)-aligned groups of 512 so the
    elementwise LSTM math is local to one [128 x 512] tile set.
  - Bias add on DVE (PSUM + replicated-bias SBUF tile), sigmoid/tanh on ACT,
    cell/hidden updates on DVE, everything overlapped with the PE stream by
    the Tile scheduler.
"""

import sys

if "/opt/trn_rl_repo" not in sys.path:
    sys.path.insert(0, "/opt/trn_rl_repo")

import ml_dtypes
import numpy as np

import concourse.bass as bass
import concourse.mybir as mybir
import concourse.tile as tile
from concourse import bacc
from concourse.bass_utils import run_bass_kernel_spmd

N_CORES = 8
B = 8192
IN_SIZE = 1024
H = 1024
D = IN_SIZE + H          # 2048 contraction dim
G4 = 4 * H               # 4096 gate columns
BC = B // N_CORES        # 1024 batch rows per core
P = 128                  # partitions
KT = D // P              # 16 k-tiles
MT = BC // P             # 8 m-tiles (batch row tiles) per core
NG = 512                 # gate columns processed per group (per gate)
HGRPS = H // NG          # 2 column groups

_NC_CACHE = {}


def _build(iters: int = 1) -> bass.Bass:
    nc = bacc.Bacc("TRN2", target_bir_lowering=False, debug=False)

    at = nc.dram_tensor("at", [D, BC], mybir.dt.bfloat16, kind="ExternalInput")
    w = nc.dram_tensor("w", [D, G4], mybir.dt.bfloat16, kind="ExternalInput")
    br = nc.dram_tensor("br", [P, G4], mybir.dt.float32, kind="ExternalInput")
    cs = nc.dram_tensor("cs", [BC, H], mybir.dt.float32, kind="ExternalInput")
    nh = nc.dram_tensor("nh", [BC, H], mybir.dt.float32, kind="ExternalOutput")
    ncl = nc.dram_tensor("ncl", [BC, H], mybir.dt.float32, kind="ExternalOutput")

    at_r = at.rearrange("(ko ki) b -> ki ko b", ki=P)   # [128, KT, BC]
    w_r = w.rearrange("(ko ki) n -> ki ko n", ki=P)     # [128, KT, G4]
    cs_r = cs.rearrange("(m p) h -> m p h", p=P)        # [MT, 128, H]
    nh_r = nh.rearrange("(m p) h -> m p h", p=P)
    ncl_r = ncl.rearrange("(m p) h -> m p h", p=P)

    AF = mybir.ActivationFunctionType

    from contextlib import ExitStack, nullcontext

    with tile.TileContext(nc) as tc:
        with (
            tc.tile_pool(name="resident", bufs=1) as rpool,
            tc.tile_pool(name="work", bufs=3) as wpool,
            tc.tile_pool(name="psum", bufs=2, space="PSUM") as ppool,
        ):
            # benchmarking mode: repeat the whole body in-NEFF via a dynamic
            # loop (no instruction growth); iters=1 emits a straight-line body
            with (tc.For_i(0, iters, 1) if iters > 1 else nullcontext()):
                at_sb = rpool.tile([P, KT, BC], mybir.dt.bfloat16, tag="at")
                w_sb = rpool.tile([P, KT, G4], mybir.dt.bfloat16, tag="w")
                br_sb = rpool.tile([P, G4], mybir.dt.float32, tag="br")
                nc.sync.dma_start(out=br_sb[:], in_=br[:])
                for k in range(KT):
                    nc.sync.dma_start(out=at_sb[:, k, :], in_=at_r[:, k, :])
                    nc.sync.dma_start(out=w_sb[:, k, :], in_=w_r[:, k, :])

                for m in range(MT):
                    for g in range(HGRPS):
                        # column starts for i, f, o, g slices of this h-group
                        cols = [q * H + g * NG for q in range(4)]
                        ps = [
                            ppool.tile([P, NG], mybir.dt.float32, tag=f"ps{q}", name=f"ps{q}")
                            for q in range(4)
                        ]
                        lhs0 = at_sb[:, 0, m * P : (m + 1) * P]
                        for k in range(KT):
                            lhs = at_sb[:, k, m * P : (m + 1) * P]
                            for q in range(4):
                                nc.tensor.matmul(
                                    ps[q][:],
                                    lhsT=lhs,
                                    rhs=w_sb[:, k, cols[q] : cols[q] + NG],
                                    start=(k == 0),
                                    stop=(k == KT - 1),
                                )
                        # bias add (DVE) + activation (ACT), PSUM -> SBUF
                        gt = [
                            wpool.tile([P, NG], mybir.dt.float32, tag=f"gt{q}", name=f"gt{q}")
                            for q in range(4)
                        ]
                        for q in range(4):
                            nc.vector.tensor_add(
                                out=gt[q][:],
                                in0=ps[q][:],
                                in1=br_sb[:, cols[q] : cols[q] + NG],
                            )
                        for q in range(3):
                            nc.scalar.activation(gt[q][:], gt[q][:], AF.Sigmoid)
                        nc.scalar.activation(gt[3][:], gt[3][:], AF.Tanh)

                        cl = wpool.tile([P, NG], mybir.dt.float32, tag="cl")
                        nc.sync.dma_start(
                            out=cl[:], in_=cs_r[m, :, g * NG : (g + 1) * NG]
                        )
                        si, sf, so, sg = gt
                        # new_cell = sigmoid(f)*c + sigmoid(i)*tanh(g)  -> sf
                        nc.vector.tensor_mul(out=sf[:], in0=sf[:], in1=cl[:])
                        nc.vector.tensor_mul(out=si[:], in0=si[:], in1=sg[:])
                        nc.vector.tensor_add(out=sf[:], in0=sf[:], in1=si[:])
                        # new_hidden = sigmoid(o)*tanh(new_cell)        -> so
                        nc.scalar.activation(sg[:], sf[:], AF.Tanh)
                        nc.vector.tensor_mul(out=so[:], in0=so[:], in1=sg[:])
                        nc.sync.dma_start(
                            out=ncl_r[m, :, g * NG : (g + 1) * NG], in_=sf[:]
                        )
                        nc.sync.dma_start(
                            out=nh_r[m, :, g * NG : (g + 1) * NG], in_=so[:]
                        )
    nc.finalize()
    return nc


def get_nc(iters: int = 1) -> bass.Bass:
    if iters not in _NC_CACHE:
        _NC_CACHE[iters] = _build(iters)
    return _NC_CACHE[iters]


def make_in_maps(input, hidden, cell_state, W_i, b_i, W_f, b_f, W_o, b_o, W_g, b_g):
    comb = np.concatenate(
        [np.asarray(input, np.float32), np.asarray(hidden, np.float32)], axis=1
    )  # [B, D]
    W = np.concatenate(
        [np.asarray(W_i), np.asarray(W_f), np.asarray(W_o), np.asarray(W_g)], axis=1
    ).astype(np.float32)  # [D, 4H]
    b = np.concatenate(
        [np.asarray(b_i), np.asarray(b_f), np.asarray(b_o), np.asarray(b_g)]
    ).astype(np.float32)  # [4H]

    at_full = comb.T.astype(ml_dtypes.bfloat16)  # [D, B]
    w_bf = np.ascontiguousarray(W.astype(ml_dtypes.bfloat16))
    br = np.ascontiguousarray(np.broadcast_to(b, (P, G4)))
    cs = np.asarray(cell_state, np.float32)

    in_maps = []
    for c in range(N_CORES):
        sl = slice(c * BC, (c + 1) * BC)
        in_maps.append(
            {
                "at": np.ascontiguousarray(at_full[:, sl]),
                "w": w_bf,
                "br": br,
                "cs": np.ascontiguousarray(cs[sl]),
            }
        )
    return in_maps


def kernel(input, hidden, cell_state, W_i, b_i, W_f, b_f, W_o, b_o, W_g, b_g):
    in_maps = make_in_maps(
        input, hidden, cell_state, W_i, b_i, W_f, b_f, W_o, b_o, W_g, b_g
    )
    nc = get_nc(1)
    res = run_bass_kernel_spmd(nc, in_maps, core_ids=list(range(N_CORES)))
    new_hidden = np.concatenate(
        [res.results[c]["nh"] for c in range(N_CORES)], axis=0
    )
    new_cell = np.concatenate(
        [res.results[c]["ncl"] for c in range(N_CORES)], axis=0
    )
    return new_hidden, new_cell


# revision 15
# speedup vs baseline: 236.7168x; 36.5386x over previous
"""Trainium2 Bass kernel for a fused CustomLSTMCell.

Math (reference):
    combined = concat([input, hidden], axis=1)            # [B, D], D = 2048
    gates    = combined @ concat([W_i,W_f,W_o,W_g], 1) + b  # [B, 4H]
    i, f, o, g = split(gates, 4, axis=1)
    new_cell   = sigmoid(f) * cell_state + sigmoid(i) * tanh(g)
    new_hidden = sigmoid(o) * tanh(new_cell)

Strategy:
  - Data-parallel over batch: 8 cores x 1024 rows each. No collectives.
  - Host prepares A^T = combined.T (contraction dim D on partitions) in bf16
    and W in bf16; PSUM accumulates in fp32.
  - Per core GEMM: [1024, 2048] @ [2048, 4096] as 128x128x512 matmul tiles;
    the stationary operand is the A^T tile (reused across 4 gate tiles per
    K step), the moving operand is W.
  - Gate columns are processed in (i,f,o,g)-aligned groups of 512 so the
    elementwise LSTM math is local to one [128 x 512] tile set.
  - Bias add on DVE (PSUM + replicated-bias SBUF tile), sigmoid/tanh on ACT,
    cell/hidden updates on DVE, everything overlapped with the PE stream by
    the Tile scheduler.
"""

import sys

if "/opt/trn_rl_repo" not in sys.path:
    sys.path.insert(0, "/opt/trn_rl_repo")

import ml_dtypes
import numpy as np

import concourse.bass as bass
import concourse.mybir as mybir
import concourse.tile as tile
from concourse import bacc
from concourse.bass_utils import run_bass_kernel_spmd

N_CORES = 8
B = 8192
IN_SIZE = 1024
H = 1024
D = IN_SIZE + H          # 2048 contraction dim
G4 = 4 * H               # 4096 gate columns
BC = B // N_CORES        # 1024 batch rows per core
P = 128                  # partitions
KT = D // P              # 16 k-tiles
MT = BC // P             # 8 m-tiles (batch row tiles) per core
NG = 512                 # gate columns processed per group (per gate)
HGRPS = H // NG          # 2 column groups

_NC_CACHE = {}


def _build(iters: int = 1, loads_in_loop: bool = True, compute=True, ng: int = NG) -> bass.Bass:
    # compute: True = full body, False = no compute, "mm" = matmuls only
    hgrps = H // min(ng, 512)
    psum_bufs = 2 if ng <= 512 else 1
    nc = bacc.Bacc("TRN2", target_bir_lowering=False, debug=False)

    at = nc.dram_tensor("at", [D, BC], mybir.dt.bfloat16, kind="ExternalInput")
    w = nc.dram_tensor("w", [D, G4], mybir.dt.bfloat16, kind="ExternalInput")
    br = nc.dram_tensor("br", [P, G4], mybir.dt.float32, kind="ExternalInput")
    cs = nc.dram_tensor("cs", [BC, H], mybir.dt.float32, kind="ExternalInput")
    nh = nc.dram_tensor("nh", [BC, H], mybir.dt.float32, kind="ExternalOutput")
    ncl = nc.dram_tensor("ncl", [BC, H], mybir.dt.float32, kind="ExternalOutput")

    at_r = at.rearrange("(ko ki) b -> ki ko b", ki=P)   # [128, KT, BC]
    w_r = w.rearrange("(ko ki) n -> ki ko n", ki=P)     # [128, KT, G4]
    cs_r = cs.rearrange("(m p) h -> m p h", p=P)        # [MT, 128, H]
    nh_r = nh.rearrange("(m p) h -> m p h", p=P)
    ncl_r = ncl.rearrange("(m p) h -> m p h", p=P)

    AF = mybir.ActivationFunctionType

    from contextlib import nullcontext

    with tile.TileContext(nc) as tc:
        with (
            tc.tile_pool(name="resident", bufs=1) as rpool,
            tc.tile_pool(name="work", bufs=3) as wpool,
            tc.tile_pool(name="psum", bufs=psum_bufs, space="PSUM") as ppool,
        ):
            # benchmarking mode: repeat the whole body in-NEFF via a dynamic
            # loop (no instruction growth); iters=1 emits a straight-line body
            def loads():
                at_sb = rpool.tile([P, KT, BC], mybir.dt.bfloat16, tag="at", name="at_sb")
                w_sb = rpool.tile([P, KT, G4], mybir.dt.bfloat16, tag="w", name="w_sb")
                br_sb = rpool.tile([P, G4], mybir.dt.float32, tag="br", name="br_sb")
                nc.sync.dma_start(out=br_sb[:], in_=br[:])
                for k in range(KT):
                    nc.sync.dma_start(out=at_sb[:, k, :], in_=at_r[:, k, :])
                    nc.sync.dma_start(out=w_sb[:, k, :], in_=w_r[:, k, :])
                return at_sb, w_sb, br_sb

            if not loads_in_loop:
                at_sb, w_sb, br_sb = loads()
            with (tc.For_i(0, iters, 1) if iters > 1 else nullcontext()):
                if loads_in_loop:
                    at_sb, w_sb, br_sb = loads()
                for m in range(MT if compute else 0):
                    mm_groups = []
                    if ng <= 512:
                        for g in range(hgrps):
                            ps = [
                                ppool.tile([P, ng], mybir.dt.float32, tag=f"ps{q}", name=f"ps{q}")
                                for q in range(4)
                            ]
                            for k in range(KT):
                                lhs = at_sb[:, k, m * P : (m + 1) * P]
                                for q in range(4):
                                    nc.tensor.matmul(
                                        ps[q][:],
                                        lhsT=lhs,
                                        rhs=w_sb[:, k, q * H + g * ng : q * H + (g + 1) * ng],
                                        start=(k == 0),
                                        stop=(k == KT - 1),
                                    )
                            # psum slices for the elementwise stage, one per gate
                            mm_groups.append((g, [t[:] for t in ps]))
                    else:
                        # one 2-bank matmul per gate covers the full H columns
                        ps = [
                            ppool.tile([P, H], mybir.dt.float32, tag=f"ps{q}", name=f"ps{q}")
                            for q in range(4)
                        ]
                        for k in range(KT):
                            lhs = at_sb[:, k, m * P : (m + 1) * P]
                            for q in range(4):
                                nc.tensor.matmul(
                                    ps[q][:],
                                    lhsT=lhs,
                                    rhs=w_sb[:, k, q * H : (q + 1) * H],
                                    start=(k == 0),
                                    stop=(k == KT - 1),
                                )
                        for g in range(2):
                            mm_groups.append(
                                (g, [t[:, g * 512 : (g + 1) * 512] for t in ps])
                            )
                    if compute == "mm":
                        continue
                    for g, psl in mm_groups:
                        eg = 512 if ng > 512 else ng
                        cols = [q * H + g * eg for q in range(4)]
                        # bias add (DVE) + activation (ACT), PSUM -> SBUF
                        gt = [
                            wpool.tile([P, eg], mybir.dt.float32, tag=f"gt{q}", name=f"gt{q}")
                            for q in range(4)
                        ]
                        for q in range(4):
                            nc.vector.tensor_add(
                                out=gt[q][:],
                                in0=psl[q],
                                in1=br_sb[:, cols[q] : cols[q] + eg],
                            )
                        for q in range(3):
                            nc.scalar.activation(gt[q][:], gt[q][:], AF.Sigmoid)
                        nc.scalar.activation(gt[3][:], gt[3][:], AF.Tanh)

                        cl = wpool.tile([P, eg], mybir.dt.float32, tag="cl")
                        nc.sync.dma_start(
                            out=cl[:], in_=cs_r[m, :, g * eg : (g + 1) * eg]
                        )
                        si, sf, so, sg = gt
                        # new_cell = sigmoid(f)*c + sigmoid(i)*tanh(g)  -> sf
                        nc.vector.tensor_mul(out=sf[:], in0=sf[:], in1=cl[:])
                        nc.vector.tensor_mul(out=si[:], in0=si[:], in1=sg[:])
                        nc.vector.tensor_add(out=sf[:], in0=sf[:], in1=si[:])
                        # new_hidden = sigmoid(o)*tanh(new_cell)        -> so
                        nc.scalar.activation(sg[:], sf[:], AF.Tanh)
                        nc.vector.tensor_mul(out=so[:], in0=so[:], in1=sg[:])
                        nc.sync.dma_start(
                            out=ncl_r[m, :, g * eg : (g + 1) * eg], in_=sf[:]
                        )
                        nc.sync.dma_start(
                            out=nh_r[m, :, g * eg : (g + 1) * eg], in_=so[:]
                        )
    nc.finalize()
    return nc


def get_nc(iters: int = 1, loads_in_loop: bool = True, compute=True, ng: int = NG) -> bass.Bass:
    key = (iters, loads_in_loop, compute, ng)
    if key not in _NC_CACHE:
        _NC_CACHE[key] = _build(iters, loads_in_loop, compute, ng)
    return _NC_CACHE[key]


def make_in_maps(input, hidden, cell_state, W_i, b_i, W_f, b_f, W_o, b_o, W_g, b_g):
    comb = np.concatenate(
        [np.asarray(input, np.float32), np.asarray(hidden, np.float32)], axis=1
    )  # [B, D]
    W = np.concatenate(
        [np.asarray(W_i), np.asarray(W_f), np.asarray(W_o), np.asarray(W_g)], axis=1
    ).astype(np.float32)  # [D, 4H]
    b = np.concatenate(
        [np.asarray(b_i), np.asarray(b_f), np.asarray(b_o), np.asarray(b_g)]
    ).astype(np.float32)  # [4H]

    at_full = comb.T.astype(ml_dtypes.bfloat16)  # [D, B]
    w_bf = np.ascontiguousarray(W.astype(ml_dtypes.bfloat16))
    br = np.ascontiguousarray(np.broadcast_to(b, (P, G4)))
    cs = np.asarray(cell_state, np.float32)

    in_maps = []
    for c in range(N_CORES):
        sl = slice(c * BC, (c + 1) * BC)
        in_maps.append(
            {
                "at": np.ascontiguousarray(at_full[:, sl]),
                "w": w_bf,
                "br": br,
                "cs": np.ascontiguousarray(cs[sl]),
            }
        )
    return in_maps


def kernel(input, hidden, cell_state, W_i, b_i, W_f, b_f, W_o, b_o, W_g, b_g):
    in_maps = make_in_maps(
        input, hidden, cell_state, W_i, b_i, W_f, b_f, W_o, b_o, W_g, b_g
    )
    nc = get_nc(1)
    res = run_bass_kernel_spmd(nc, in_maps, core_ids=list(range(N_CORES)))
    new_hidden = np.concatenate(
        [res.results[c]["nh"] for c in range(N_CORES)], axis=0
    )
    new_cell = np.concatenate(
        [res.results[c]["ncl"] for c in range(N_CORES)], axis=0
    )
    return new_hidden, new_cell
